# revision 8
# baseline (speedup 1.0000x reference)
"""Trainium2 Bass kernel for nn_CostLearning quadratic cost:

    cost[i] = sum_d exp(q_diag_log[d]) * states[i,d]^2
            + sum_d exp(r_diag_log[d]) * actions[i,d]^2

Sharding: pure data parallel over B*T rows across 8 NeuronCores.
Per core: rows are laid out so SBUF partition p owns 256 *consecutive*
rows of the core's shard -> every DMA is 128 partitions x large
contiguous runs, and the d-reduction is a free-axis (X) segmented
reduce on the vector engine.

v2 layout (from trace analysis of v1 @ 68.1us):
  exec window = main_start .. last_teardown_event. v1 breakdown:
  2.2us first-DMA latency + 50.2us stream (SDMA ~100% busy @ ~400GB/s)
  + 5.3us compute drain + 1.8us store + 8.6us sem-teardown ladder.
  v2 attacks the non-stream parts:
  - tapered chunks: big (64 rows/partition, 4MB) early for low
    instruction count, tiny (4 rows) at the end so the post-stream
    drain is ~1.5us instead of 7us
  - squares written as bf16 (inputs stay f32): DVE segmented reduce
    runs at 2x, so the last chunk's reduce is short; f32 accumulate
    in the reduce keeps error ~1e-3 << 2e-2 gate
  - output stored in 5 pieces, 4 of them mid-stream; only a 16-row
    (8KB) store remains after the last input chunk
  - fewer total instructions -> shorter end-of-kernel semaphore
    teardown ladder
"""

import numpy as np

B, T, DS, DA = 128, 2048, 128, 32
BT = B * T
NCORES = 8
RPC = BT // NCORES        # rows per core = 32768
P = 128                   # SBUF partitions
NPP = RPC // P            # rows per partition = 256

# states chunks: uniform 1MB (16 rows/partition) for a fine-grained
# DMA/ACT/DVE pipeline, tapered tail (8,4,4) so the post-stream
# square+reduce drain is tiny. Lumpy 4MB chunks measurably stall the
# tail (v3: big reduces serialize ahead of the small tail chunks).
S_SCHED = [16] * 15 + [8, 4, 4]
assert sum(S_SCHED) == NPP

_cache = {}


def _build(weighted: bool):
    import concourse.bacc as bacc
    import concourse.bass as bass
    import concourse.tile as tile
    from concourse import mybir

    f32 = mybir.dt.float32
    bf16 = mybir.dt.bfloat16
    # all-bf16 intermediates: DVE's 2x_1P perf mode (2 elem/cycle)
    # requires every src AND dst of tensor_reduce to be 2-byte. The
    # reduce ALU accumulates in f32 internally; only the final write
    # rounds to bf16, so the error is ~2^-9 per cost term, well under
    # the 2e-2 gate.
    sq_dt = f32 if weighted else bf16
    red_dt = f32 if weighted else bf16
    nc = bacc.Bacc("TRN2", target_bir_lowering=False, debug=False)

    states = nc.dram_tensor("states", [RPC, DS], f32, kind="ExternalInput")
    actions = nc.dram_tensor("actions", [RPC, DA], f32, kind="ExternalInput")
    if weighted:
        qlog = nc.dram_tensor("qlog", [DS], f32, kind="ExternalInput")
        rlog = nc.dram_tensor("rlog", [DA], f32, kind="ExternalInput")
    cost = nc.dram_tensor("cost", [RPC], f32, kind="ExternalOutput")

    # partition p owns shard rows [p*NPP, (p+1)*NPP)
    sview = states[:].rearrange("(p n) d -> p n d", p=P)    # [128, 256, 128]
    aview = actions[:].rearrange("(p n) d -> p n d", p=P)   # [128, 256, 32]
    oview = cost[:].rearrange("(p n) -> p n", p=P)          # [128, 256]

    s_max = max(S_SCHED)
    a_max = 64

    with tile.TileContext(nc) as tc:
        with (
            tc.tile_pool(name="sio", bufs=6) as sio,
            tc.tile_pool(name="ssqp", bufs=4) as ssqp,
            tc.tile_pool(name="aio", bufs=3) as aio,
            tc.tile_pool(name="asqp", bufs=3) as asqp,
            tc.tile_pool(name="accp", bufs=1) as accp,
        ):
            st_red = accp.tile([P, NPP], red_dt)
            ac_red = accp.tile([P, NPP], red_dt)
            out_t = accp.tile([P, NPP], f32)

            if weighted:
                # exp(weights), broadcast to all partitions and tiled
                # along the free axis to match one chunk's [P, n, d]
                qrep = accp.tile([P, s_max, DS], f32)
                rrep = accp.tile([P, a_max, DA], f32)
                qap = qlog[:]
                rap = rlog[:]
                qb = bass.AP(tensor=qap.tensor, offset=qap.offset,
                             ap=[[0, P], [0, s_max], [1, DS]])
                rb = bass.AP(tensor=rap.tensor, offset=rap.offset,
                             ap=[[0, P], [0, a_max], [1, DA]])
                nc.gpsimd.dma_start(out=qrep, in_=qb)
                nc.gpsimd.dma_start(out=rrep, in_=rb)
                nc.scalar.activation(qrep, qrep,
                                     mybir.ActivationFunctionType.Exp)
                nc.scalar.activation(rrep, rrep,
                                     mybir.ActivationFunctionType.Exp)

            def do_schunk(row0, n):
                s_t = sio.tile([P, s_max, DS], f32, name="s_t")
                nc.sync.dma_start(out=s_t[:, :n, :],
                                  in_=sview[:, row0:row0 + n, :])
                ssq = ssqp.tile([P, s_max, DS], sq_dt, name="ssq")
                nc.scalar.activation(ssq[:, :n, :], s_t[:, :n, :],
                                     mybir.ActivationFunctionType.Square)
                if weighted:
                    nc.vector.tensor_mul(ssq[:, :n, :], ssq[:, :n, :],
                                         qrep[:, :n, :])
                with nc.allow_low_precision("bf16 cost partials; gate is 2e-2"):
                    nc.vector.reduce_sum(
                        out=st_red[:, row0:row0 + n],
                        in_=ssq[:, :n, :],
                        axis=mybir.AxisListType.X,
                    )

            def do_achunk(row0, n):
                a_t = aio.tile([P, a_max, DA], f32, name="a_t")
                nc.sync.dma_start(out=a_t[:, :n, :],
                                  in_=aview[:, row0:row0 + n, :])
                asq = asqp.tile([P, a_max, DA], sq_dt, name="asq")
                nc.scalar.activation(asq[:, :n, :], a_t[:, :n, :],
                                     mybir.ActivationFunctionType.Square)
                if weighted:
                    nc.vector.tensor_mul(asq[:, :n, :], asq[:, :n, :],
                                         rrep[:, :n, :])
                with nc.allow_low_precision("bf16 cost partials; gate is 2e-2"):
                    nc.vector.reduce_sum(
                        out=ac_red[:, row0:row0 + n],
                        in_=asq[:, :n, :],
                        axis=mybir.AxisListType.X,
                    )

            def finalize(r0, r1, store0=None, last=False):
                # add this region; store is a (row0, row1) range that may
                # cover several finalized regions. Mid-stream stores go on
                # the idle gpsimd (SWDGE) queue: HWDGE rings drain FIFO
                # per issuing engine, so a compute-gated store on the sync
                # ring would stall every later input DMA behind it. The
                # final store uses sync (lower latency; ring is empty by
                # then).
                nc.vector.tensor_add(out_t[:, r0:r1], st_red[:, r0:r1],
                                     ac_red[:, r0:r1])
                if store0 is not None:
                    eng = nc.sync if last else nc.gpsimd
                    eng.dma_start(out=oview[:, store0:r1],
                                  in_=out_t[:, store0:r1])

            # explicit interleaved emission: 1MB states chunks drive the
            # stream; action chunks and finalize/stores slot in as their
            # row ranges complete.
            do_schunk(0, 16); do_schunk(16, 16)
            do_achunk(0, 64)
            do_schunk(32, 16); do_schunk(48, 16)
            do_schunk(64, 16); do_schunk(80, 16)
            finalize(0, 64, store0=0)
            do_achunk(64, 64)
            do_schunk(96, 16); do_schunk(112, 16)
            do_schunk(128, 16); do_schunk(144, 16)
            finalize(64, 128, store0=64)
            do_achunk(128, 64)
            do_schunk(160, 16); do_schunk(176, 16)
            do_schunk(192, 16); do_schunk(208, 16)
            finalize(128, 192, store0=128)
            do_achunk(192, 32)
            do_schunk(224, 16)
            do_achunk(224, 16)
            finalize(192, 240, store0=192)
            do_schunk(240, 8)
            do_achunk(240, 16)
            do_schunk(248, 4)
            do_schunk(252, 4)
            finalize(240, 256, store0=240, last=True)

    nc.compile()
    return nc


def _get_program(weighted: bool):
    if weighted not in _cache:
        _cache[weighted] = _build(weighted)
    return _cache[weighted]


def _run(states2d, actions2d, q, r, weighted, trace=False):
    from concourse.bass_utils import run_bass_kernel_spmd

    nc = _get_program(weighted)
    in_maps = []
    for c in range(NCORES):
        m = {
            "states": states2d[c * RPC:(c + 1) * RPC],
            "actions": actions2d[c * RPC:(c + 1) * RPC],
        }
        if weighted:
            m["qlog"] = q
            m["rlog"] = r
        in_maps.append(m)
    res = run_bass_kernel_spmd(nc, in_maps, list(range(NCORES)), trace=trace)
    out = np.concatenate([np.asarray(res.results[c]["cost"]) for c in range(NCORES)])
    return out.astype(np.float32, copy=False), res


def kernel(states, actions, q_diag_log, r_diag_log):
    states2d = np.ascontiguousarray(np.asarray(states, dtype=np.float32)).reshape(BT, DS)
    actions2d = np.ascontiguousarray(np.asarray(actions, dtype=np.float32)).reshape(BT, DA)
    q = np.ascontiguousarray(np.asarray(q_diag_log, dtype=np.float32))
    r = np.ascontiguousarray(np.asarray(r_diag_log, dtype=np.float32))
    weighted = bool(np.any(q != 0.0) or np.any(r != 0.0))
    out, _ = _run(states2d, actions2d, q, r, weighted)
    return out
